# revision 1
# baseline (speedup 1.0000x reference)
"""Chamfer distance kernel for Trainium2 (8 NeuronCores, data-parallel over batch).

reference:
    dist[b,i,j] = |x_bi|^2 + |y_bj|^2 - 2<x_bi, y_bj>
    out = mean_b,j( min_i dist ) + mean_b,i( min_j dist )

Device algorithm (per core = one batch):
  The tensor engine produces raw distance blocks directly with K=5 matmuls
  over augmented features:
      lhsT of a point p:  (p0, p1, p2, |p|^2, 1)     [5, 128] chunks
      rhs  of a point q:  (-2q0, -2q1, -2q2, 1, |q|^2) [5, 512] chunks
      => lhsT.T @ rhs = dist block [128, 512] in PSUM (fp32).
  Per pair of blocks (same 128 points x 1024 opposite points): ScalarE copies
  one block PSUM->SBUF; VectorE runs one tensor_tensor_scan(min, min) over
  (psum_block, sbuf_block) whose running state is the min over both streams;
  chained via initial=prev[:, -1:] across the whole row, so one final [128,1]
  copy per 128-point group yields min over ALL opposite points. Two
  orientations (x-on-partitions -> dist2, y-on-partitions -> dist1) keep both
  reductions free-axis only. Host sums the [128, n_groups] strip.

  Matmuls are packed 2-way into PE row-groups 0 and 1 (K=5 each, via
  tile_position) so weight loads overlap matmul streaming.
"""

import numpy as np

import concourse.bass as bass
import concourse.tile as tile
import concourse.mybir as mybir
from concourse.bass_utils import run_bass_kernel_spmd
from concourse.vector_clock import ScopedClock

B, N, M, D = 8, 8192, 8192, 3
N_CORES = 8
FD = 512  # free-dim elements per block = one PSUM bank of fp32


# --- workaround: this walrus build accepts only 1 sync-wait per instruction;
# split excess waits onto single-wait NoOps emitted on the same engine just
# before the offending instruction (per-engine program order preserves the
# semantics: all waits complete before the instruction issues).
_orig_add_instruction = tile.TileContext._add_instruction


def _add_instruction_split(self, inst):
    si = inst.sync_info
    if si is not None and len(si.on_wait) > 1:
        waits = list(si.on_wait)
        inst.sync_info = mybir.SyncInfo(on_wait=[waits[-1]], on_update=list(si.on_update))
        eng = self.nc.engines[inst.engine]
        for w in waits[:-1]:
            nop = eng.nop(nofuse=True)
            nop.ins.sync_info = mybir.SyncInfo(on_wait=[w], on_update=[])
    _orig_add_instruction(self, inst)


tile.TileContext._add_instruction = _add_instruction_split


def _drain_and_barrier_split(self, tick_clock, wait_clock):
    nc = self.nc
    probe = nc.sync.nop(nofuse=True)
    wait_clock.add_sem_waits(probe.ins, ScopedClock({None: tick_clock.global_clock}))
    si = probe.ins.sync_info
    waits = list(si.on_wait) if si is not None else []
    upds = list(si.on_update) if si is not None else []
    probe.ins.sync_info = mybir.SyncInfo(on_wait=waits[:1], on_update=upds)
    for w in waits[1:]:
        nop = nc.sync.nop(nofuse=True)
        nop.ins.sync_info = mybir.SyncInfo(on_wait=[w], on_update=[])
    nc.sync.drain()
    nc.all_engine_barrier()
    assert self.sems is not None
    popped = nc._tile_sem_poison_stack.pop()
    assert popped is self._sem_poison
    nc.clear_and_free_semaphores(list(self.sems.allocated().values()))
    nc.all_engine_barrier()


tile.TileContext._drain_and_barrier = _drain_and_barrier_split


# --- enable walrus LDWEIGHTS dedup: our inner loop issues 16 consecutive
# matmuls with identical stationary weights; without ldw-opt each fp32 matmul
# re-streams its weight load, which dominates PE time at K=5.
import concourse.bass_utils as _bass_utils

_orig_run_command = _bass_utils.run_command


def _run_command_ldwopt(argv, **kwargs):
    argv = ["--enable-ldw-opt=true" if a == "--enable-ldw-opt=false" else a
            for a in argv]
    return _orig_run_command(argv, **kwargs)


_bass_utils.run_command = _run_command_ldwopt


def build_nc(n=N, m=M, repeat=1):
    """Bass program for one core: one batch of chamfer(n x-points, m y-points).

    Inputs (per orientation o in {a: x-partitions, b: y-partitions}):
      l{o}0, l{o}1: [5, n/2] lhsT feature halves (strip 0 / strip 1)
      r_{o}: [5, m] rhs point features
    Output: strip [128, 2 * n_xb]; strip[p, o*n_xb + s*n_xb/2 + xb] = min over
    all opposite-side points for point index s*(n/2) + xb*128 + p.
    """
    assert n == m, "loop/strip layout assumes equal point counts"
    assert n % 512 == 0 and m % (2 * FD) == 0
    dt = mybir.dt.float32
    n_xb = n // 128          # 128-point blocks on the partition side
    n_xb_q = n_xb // 4       # per strip (4 row-group strips)
    n_sp = m // (2 * FD)     # block pairs along the free side
    BIG = 3.0e38

    nc = bass.Bass()
    params = {}
    for o in ("a", "b"):
        params[f"l_{o}"] = nc.declare_dram_parameter(f"l_{o}", [4, 5, n // 4], dt, isOutput=False)
        params[f"r_{o}"] = nc.declare_dram_parameter(f"r_{o}", [5, m], dt, isOutput=False)
    out = nc.declare_dram_parameter("strip", [128, 2 * n_xb], dt, isOutput=True)

    with tile.TileContext(nc) as tc:
        with (
            tc.tile_pool(name="inputs", bufs=1) as in_pool,
            tc.tile_pool(name="p_psum", bufs=4, space="PSUM") as p_pool,
            tc.tile_pool(name="q_psum", bufs=2, space="PSUM") as q_pool,
            tc.tile_pool(name="qsb", bufs=4) as qsb_pool,
            tc.tile_pool(name="scan", bufs=8) as scan_pool,
            tc.tile_pool(name="strip", bufs=1) as strip_pool,
        ):
            sb = {}
            for o in ("a", "b"):
                lt = in_pool.tile([101, n // 4], dt, tag=f"l_{o}")
                rt = in_pool.tile([101, m], dt, tag=f"r_{o}")
                for s in range(4):
                    nc.sync.dma_start(lt[32 * s:32 * s + 5, :], params[f"l_{o}"][s])
                    nc.sync.dma_start(rt[32 * s:32 * s + 5, :], params[f"r_{o}"][:])
                sb[o] = (lt, rt)

            strip_t = strip_pool.tile([128, 2 * n_xb], dt)

            for _rep in range(repeat):
              for oi, o in enumerate(("a", "b")):
                lt, rt = sb[o]
                for xb in range(n_xb_q):
                    prev = [None] * 4
                    for sp in range(n_sp):
                        c0 = (2 * sp) * FD
                        c1 = (2 * sp + 1) * FD

                        def lhs(s):
                            p0 = 32 * s
                            return lt[p0:p0 + 5, xb * 128:(xb + 1) * 128]

                        # q blocks first (copy sources), paired two strips per
                        # 2-bank PSUM tile so ACT does two FD=1024 copies.
                        qps = [q_pool.tile([128, 2 * FD], dt, name=f"qp{h}",
                                           tag="q") for h in range(2)]
                        for s in range(4):
                            p0 = 32 * s
                            nc.tensor.matmul(
                                qps[s // 2][:, (s % 2) * FD:(s % 2 + 1) * FD],
                                lhs(s), rt[p0:p0 + 5, c1:c1 + FD],
                                start=True, stop=True, tile_position=(p0, 0))
                        qsbs = []
                        for h in range(2):
                            q_sb = qsb_pool.tile([128, 2 * FD], dt,
                                                 name=f"qs{h}", tag="qs")
                            nc.scalar.copy(q_sb[:], qps[h][:])
                            qsbs.append(q_sb)
                        pts = []
                        for s in range(4):
                            p0 = 32 * s
                            p_t = p_pool.tile([128, FD], dt, name=f"p{s}", tag="p")
                            nc.tensor.matmul(
                                p_t[:], lhs(s), rt[p0:p0 + 5, c0:c0 + FD],
                                start=True, stop=True, tile_position=(p0, 0))
                            pts.append(p_t)
                        for s in range(4):
                            sc_t = scan_pool.tile([128, FD], dt, name=f"sc{s}", tag="sc")
                            nc.vector.tensor_tensor_scan(
                                sc_t[:], pts[s][:],
                                qsbs[s // 2][:, (s % 2) * FD:(s % 2 + 1) * FD],
                                initial=(BIG if prev[s] is None
                                         else prev[s][:, FD - 1:FD]),
                                op0=mybir.AluOpType.min,
                                op1=mybir.AluOpType.min)
                            prev[s] = sc_t
                    for s in range(4):
                        col = oi * n_xb + s * n_xb_q + xb
                        nc.scalar.copy(strip_t[:, col:col + 1],
                                       prev[s][:, FD - 1:FD])

            nc.sync.dma_start(out[:], strip_t[:])
    return nc


def _lhs_features(pts):
    """pts [n,3] float64 -> [5, n] float32: (p0,p1,p2,|p|^2,1)."""
    sq = np.sum(pts * pts, axis=-1)
    f = np.stack([pts[:, 0], pts[:, 1], pts[:, 2], sq, np.ones_like(sq)])
    return np.ascontiguousarray(f, np.float32)


def _rhs_features(pts):
    """pts [m,3] float64 -> [5, m] float32: (-2q0,-2q1,-2q2,1,|q|^2)."""
    sq = np.sum(pts * pts, axis=-1)
    f = np.stack([-2.0 * pts[:, 0], -2.0 * pts[:, 1], -2.0 * pts[:, 2],
                  np.ones_like(sq), sq])
    return np.ascontiguousarray(f, np.float32)


def make_in_map(xb, yb):
    """Per-core input map from one batch xb [n,3], yb [m,3]."""
    xb = np.asarray(xb, np.float64)
    yb = np.asarray(yb, np.float64)
    n, m = xb.shape[0], yb.shape[0]
    la = _lhs_features(xb)   # [5, n]
    lb = _lhs_features(yb)
    return {
        "l_a": np.ascontiguousarray(la.reshape(5, 4, n // 4).transpose(1, 0, 2)),
        "l_b": np.ascontiguousarray(lb.reshape(5, 4, m // 4).transpose(1, 0, 2)),
        "r_a": _rhs_features(yb),
        "r_b": _rhs_features(xb),
    }


_NC_CACHE = {}


def _get_nc(n, m):
    key = (n, m)
    if key not in _NC_CACHE:
        _NC_CACHE[key] = build_nc(n, m)
    return _NC_CACHE[key]


def run_device(x, y, trace=False):
    """x [B,n,3], y [B,m,3] -> BassKernelResults with per-core strips."""
    n, m = x.shape[1], y.shape[1]
    assert x.shape[0] == N_CORES and y.shape[0] == N_CORES
    nc = _get_nc(n, m)
    in_maps = [make_in_map(x[b], y[b]) for b in range(x.shape[0])]
    return run_bass_kernel_spmd(nc, in_maps, list(range(N_CORES)), trace=trace)


def kernel(x, y):
    x = np.asarray(x)
    y = np.asarray(y)
    n, m = x.shape[1], y.shape[1]
    n_xb = n // 128
    res = run_device(x, y)
    s2_tot = 0.0  # sum over per-x mins  (reference dist2, min over j)
    s1_tot = 0.0  # sum over per-y mins  (reference dist1, min over i)
    for b in range(x.shape[0]):
        strip = res.results[b]["strip"].astype(np.float64)
        s2_tot += strip[:, :n_xb].sum()
        s1_tot += strip[:, n_xb:].sum()
    out = s1_tot / (x.shape[0] * m) + s2_tot / (x.shape[0] * n)
    return np.float32(out)

